# revision 5
# baseline (speedup 1.0000x reference)
"""Single-head causal attention with RoPE on 8 TRN2 NeuronCores.

Sharding: core c -> batch c//2, parity p = c%2 takes the interleaved
512-row q-blocks {p, p+2, p+4, p+6} of T=4096 (causal load balance).
Each core computes full K/V for its batch; no collectives.

~144us vs the 214us fp32 baseline:
- All inputs bf16, x host-packed; first two groups strip-split across
  DMA engines (one dma_start is served by a single ~20GB/s engine).
- Q^T/K^T projected directly in [d, t] layout (weights-stationary
  matmuls); RoPE on DVE via partition-offset rotate-half; V via PE
  transpose to [s, d]; per-slot tiles for fine-grained dependencies.
- Fused phases: attention score matmuls + exps + row-sum accumulation
  stream as filler between projection matmuls as soon as their
  (q-group, k-group) dependencies complete, parking probabilities in
  SBUF; each q-block's AV matmuls + output run as one burst after its
  last half arrives. This hides the exp/rowsum cost entirely under
  the PE-bound projection phase.
- Causal-exact shrinking diagonal tiles; exp bias trick for the
  other-parity tail block; output written unnormalized [d, q] + row
  sums, with the final divide and transpose on the host.
"""
import numpy as np
import ml_dtypes

B, T, C, HD = 4, 4096, 2048, 128
P = 128
NB = 8
BS = 512
NCH = 16
SCALE = float(C) ** -0.5
NEG = -1.0e9
BF = ml_dtypes.bfloat16


def build():
    import concourse.bass as bass
    import concourse.mybir as mybir
    import bass_rust
    from concourse.tile import TileContext
    from concourse.masks import make_identity

    f32 = mybir.dt.float32
    bf16 = mybir.dt.bfloat16
    EXP = mybir.ActivationFunctionType.Exp

    nc = bass.Bass()
    xg = nc.declare_dram_parameter("xg", [NB * P, NCH * BS], bf16, isOutput=False)
    wk = nc.declare_dram_parameter("wk", [P, NCH * P], bf16, isOutput=False)
    wv = nc.declare_dram_parameter("wv", [P, NCH * P], bf16, isOutput=False)
    wq = nc.declare_dram_parameter("wq", [P, NCH * P], bf16, isOutput=False)
    cs2 = nc.declare_dram_parameter("cs2", [P, T], bf16, isOutput=False)
    sn2 = nc.declare_dram_parameter("sn2", [P, T], bf16, isOutput=False)
    tailb = nc.declare_dram_parameter("tailb", [P, 1], f32, isOutput=False)
    oT = nc.declare_dram_parameter("oT", [P, 4 * BS], f32, isOutput=True)
    smv = nc.declare_dram_parameter("smv", [1, 4 * BS], f32, isOutput=True)

    NQ = 4
    CPQ = NCH // NQ

    with TileContext(nc) as tc:
        with (
            tc.tile_pool(name="const", bufs=1) as cp,
            tc.tile_pool(name="store", bufs=1) as stp,
            tc.tile_pool(name="pt", bufs=68) as ptp,
            tc.tile_pool(name="pac", bufs=4) as pap,
            tc.tile_pool(name="osb", bufs=2) as osb,
            tc.tile_pool(name="xp", bufs=2) as xp,
            tc.tile_pool(name="rp", bufs=2) as rp,
            tc.tile_pool(name="prj", bufs=1, space="PSUM") as prj,
            tc.tile_pool(name="sps", bufs=2, space="PSUM") as sps,
            tc.tile_pool(name="o2ps", bufs=1, space="PSUM") as o2ps,
            tc.tile_pool(name="smps", bufs=1, space="PSUM") as smps,
        ):
            wkt = cp.tile([P, NCH * P], bf16, tag="wkt")
            nc.sync.dma_start(wkt[:, 0:NCH * P // 2], wk[:, 0:NCH * P // 2])
            nc.sync.dma_start(wkt[:, NCH * P // 2:], wk[:, NCH * P // 2:])
            wvt = cp.tile([P, NCH * P], bf16, tag="wvt")
            nc.sync.dma_start(wvt[:, 0:NCH * P // 2], wv[:, 0:NCH * P // 2])
            nc.sync.dma_start(wvt[:, NCH * P // 2:], wv[:, NCH * P // 2:])
            wqt = cp.tile([P, NCH * P], bf16, tag="wqt")
            nc.sync.dma_start(wqt[:, 0:NCH * P // 2], wq[:, 0:NCH * P // 2])
            nc.sync.dma_start(wqt[:, NCH * P // 2:], wq[:, NCH * P // 2:])
            cst = cp.tile([P, T], bf16, tag="cst")
            nc.scalar.dma_start(cst[:, 0:T // 2], cs2[:, 0:T // 2])
            nc.scalar.dma_start(cst[:, T // 2:], cs2[:, T // 2:])
            snt = cp.tile([P, T], bf16, tag="snt")
            nc.scalar.dma_start(snt[:, 0:T // 2], sn2[:, 0:T // 2])
            nc.scalar.dma_start(snt[:, T // 2:], sn2[:, T // 2:])
            tb = cp.tile([P, 1], f32, tag="tb")
            nc.scalar.dma_start(tb[:], tailb[:])

            ident = cp.tile([P, P], bf16, tag="ident")
            make_identity(nc, ident[:])
            tri = cp.tile([P, P], bf16, tag="tri")
            nc.gpsimd.memset(tri[:], 0.0)
            nc.gpsimd.affine_select(
                out=tri[:], in_=tri[:],
                compare_op=mybir.AluOpType.is_gt,
                fill=1.0, base=0,
                pattern=[[-1, P]], channel_multiplier=1,
            )
            ones = cp.tile([P, 1], bf16, tag="ones")
            nc.gpsimd.memset(ones[:], 1.0)

            kTs = [stp.tile([P, BS], bf16, tag=f"kT{s}", name=f"kTs{s}") for s in range(8)]
            vsbs = [stp.tile([P, BS], bf16, tag=f"vs{s}", name=f"vsbs{s}") for s in range(8)]
            qTs = [stp.tile([P, BS], bf16, tag=f"qT{s}", name=f"qTs{s}") for s in range(4)]

            # ---------- attention pair bookkeeping ----------
            # per j: halves in order; each half:
            #   (si, kind, st, off, w)   off/w only shrink for diag
            def j_halves(j):
                hs = []
                for st in range(4):
                    hs.append((j, "diag", st, st * P, BS - st * P))
                for s in list(range(j)) + [4 + s for s in range(j)]:
                    for st in range(4):
                        hs.append((s, "full", st, 0, BS))
                for st in range(4):
                    hs.append((4 + j, "tail", st, 0, BS))
                return hs

            # group position in plain order = group index
            def ready_pos(j, si):
                return max(j, si)   # needs qTs[j] (group j), kTs/vsbs[si]

            pending = []    # (j, half_idx, half_desc) in j-major order
            for j in range(4):
                for hi, h in enumerate(j_halves(j)):
                    pending.append([j, hi, h, False])   # emitted flag
            parked = {j: [] for j in range(4)}   # (half_desc, Pt tile)
            pacs = {}
            npend = {j: len(j_halves(j)) for j in range(4)}
            avdone = set()
            done_groups = 0    # groups fully emitted so far

            neng = [0]

            def emit_half(j, hi, h):
                si, kind, st, off, w = h
                scol = st * P
                Sp = sps.tile([P, BS], f32, tag="S")
                Pt = ptp.tile([P, BS], bf16, tag="Pt")
                nc.tensor.matmul(
                    Sp[:, 0:w], kTs[si][:, scol:scol + P],
                    qTs[j][:, off:BS], start=True, stop=True)
                bias = tb[:, 0:1] if kind == "tail" else 0.0
                nc.scalar.activation(Pt[:, 0:w], Sp[:, 0:w], EXP,
                                     bias=bias, scale=SCALE)
                if kind == "diag":
                    nc.vector.tensor_mul(Pt[:, 0:P], Pt[:, 0:P], tri[:])
                # row-sum accumulate into pac(j): alternate DVE/GpSimd
                if j not in pacs:
                    pacs[j] = pap.tile([P, BS], f32, tag="pac", name=f"pac{j}")
                    assert kind == "diag" and st == 0
                    nc.vector.tensor_copy(pacs[j][:], Pt[:, 0:BS])
                else:
                    nc.vector.tensor_add(pacs[j][:, off:BS],
                                         pacs[j][:, off:BS], Pt[:, 0:w])
                parked[j].append((h, Pt))

            def filler():
                # emit one ready, unemitted attention half
                for ent in pending:
                    j, hi, h, em = ent
                    if em or j in avdone:
                        continue
                    if ready_pos(j, h[0]) < done_groups:
                        ent[3] = True
                        emit_half(j, hi, h)
                        return True
                return False

            def flush_ready_js():
                for j in range(4):
                    if j in avdone or len(parked[j]) < npend[j]:
                        continue
                    avdone.add(j)
                    o2 = o2ps.tile([P, BS], f32, tag="o2")
                    nh = len(parked[j])
                    for ix, (h, Pt) in enumerate(parked[j]):
                        si, kind, st, off, w = h
                        scol = st * P
                        nc.tensor.matmul(
                            o2[:, off:BS], vsbs[si][:, scol:scol + P],
                            Pt[:, 0:w],
                            start=(ix == 0), stop=(ix == nh - 1))
                    parked[j].clear()
                    pacbf = pap.tile([P, BS], bf16, tag="pacbf")
                    nc.scalar.copy(pacbf[:], pacs[j][:])
                    sm = smps.tile([1, BS], f32, tag="sm")
                    nc.tensor.matmul(sm[:], ones[:], pacbf[:],
                                     start=True, stop=True)
                    qsl = slice(j * BS, (j + 1) * BS)
                    o2sb = osb.tile([P, BS], f32, tag="o2sb")
                    nc.vector.tensor_copy(o2sb[:], o2[:])
                    nc.sync.dma_start(oT[:, qsl], o2sb[:])
                    smsb = osb.tile([1, BS], f32, tag="smsb")
                    nc.scalar.copy(smsb[:], sm[:])
                    nc.sync.dma_start(smv[:, qsl], smsb[:])

            # ---------- fused main loop ----------
            for g in range(NB):
                gs = slice(g * BS, (g + 1) * BS)
                xts = []
                for q4 in range(NQ):
                    xt = xp.tile([P, CPQ * BS], bf16, tag=f"xg{q4}")
                    c0 = q4 * CPQ * BS
                    if g < 2:
                        for s4 in range(4):
                            r0 = g * P + s4 * 32
                            eng = nc.sync if s4 % 2 == 0 else nc.scalar
                            eng.dma_start(
                                xt[s4 * 32:(s4 + 1) * 32, :],
                                xg[r0:r0 + 32, c0:c0 + CPQ * BS])
                    else:
                        nc.sync.dma_start(
                            xt[:], xg[g * P:(g + 1) * P, c0:c0 + CPQ * BS])
                    xts.append(xt)

                def proj(wt, tag):
                    pp = prj.tile([P, BS], f32, tag=tag)
                    for ci in range(NCH):
                        nc.tensor.matmul(
                            pp[:], wt[:, ci * P:(ci + 1) * P],
                            xts[ci // CPQ][:, (ci % CPQ) * BS:
                                           (ci % CPQ + 1) * BS],
                            start=(ci == 0), stop=(ci == NCH - 1))
                        if ci % 4 == 3:
                            filler()
                    return pp

                def rope(pp, dst):
                    H = 64
                    m1 = rp.tile([P, BS], bf16, tag="m1")
                    nc.vector.tensor_mul(m1[:], pp[:], cst[:, gs])
                    rot = rp.tile([P, BS], bf16, tag="rot")
                    nc.vector.tensor_mul(rot[0:H, :], pp[H:P, :],
                                         snt[0:H, gs])
                    nc.vector.tensor_mul(rot[H:P, :], pp[0:H, :],
                                         snt[H:P, gs])
                    nc.vector.tensor_add(dst, m1[:], rot[:])

                kp = proj(wkt, "kps")
                rope(kp, kTs[g][:])
                vp = proj(wvt, "vps")
                vsbh = rp.tile([P, BS], bf16, tag="vsbh")
                nc.scalar.copy(vsbh[:], vp[:])
                vtp = prj.tile([P, BS], bf16, tag="vtp")
                for k in range(4):
                    ks = slice(k * P, (k + 1) * P)
                    nc.tensor.transpose(vtp[:, ks], vsbh[:, ks], ident[:])
                    nc.scalar.copy(vsbs[g][:, ks], vtp[:, ks])
                    filler()
                if g < 4:
                    qp = proj(wqt, "qps")
                    rope(qp, qTs[g][:])
                done_groups = g + 1
                # drain a few extra fillers between groups, then AV bursts
                for _ in range(6):
                    if not filler():
                        break
                flush_ready_js()

            # tail: whatever is left (j3's slot-7 halves + AV bursts)
            while filler():
                pass
            flush_ready_js()

    bass_rust.generate_event_semaphores(nc)
    return nc


_CACHE = {}


def _get_nc():
    if "nc" not in _CACHE:
        _CACHE["nc"] = build()
    return _CACHE["nc"]


def _prep_inputs(x, Wq, Wk, Wv, cos, sin):
    perm = np.concatenate([np.arange(0, HD, 2), np.arange(1, HD, 2)])

    def packw(wt):
        return np.ascontiguousarray(
            wt.reshape(NCH, P, HD).transpose(1, 0, 2).reshape(P, NCH * HD))

    wq = packw(Wq[perm].T.astype(BF))
    wk = packw(Wk[perm].T.astype(BF))
    wv = packw(Wv.T.astype(BF))
    cosT = cos.T.astype(np.float32)
    sinT = sin.T.astype(np.float32)
    cs2f = np.concatenate([cosT, cosT], axis=0)
    sn2f = np.concatenate([-sinT, sinT], axis=0)
    in_maps = []
    orders = []
    for c in range(8):
        b, par = c // 2, c % 2
        order = [par, par + 2, par + 4, par + 6,
                 1 - par, 3 - par, 5 - par, 7 - par]
        orders.append(order)
        xb = np.asarray(x[b], np.float32)
        xgl = np.empty((NB, P, NCH, BS), BF)
        c2 = np.empty((P, T), BF)
        s2 = np.empty((P, T), BF)
        for sl, ab in enumerate(order):
            seg = xb[ab * BS:(ab + 1) * BS].T.astype(BF)
            xgl[sl] = seg.reshape(NCH, P, BS).transpose(1, 0, 2)
            dst = slice(sl * BS, (sl + 1) * BS)
            src = slice(ab * BS, (ab + 1) * BS)
            c2[:, dst] = cs2f[:, src].astype(BF)
            s2[:, dst] = sn2f[:, src].astype(BF)
        tb = np.full((P, 1), NEG if par == 0 else 0.0, np.float32)
        in_maps.append({
            "xg": np.ascontiguousarray(xgl.reshape(NB * P, NCH * BS)),
            "wk": wk, "wv": wv, "wq": wq,
            "cs2": np.ascontiguousarray(c2),
            "sn2": np.ascontiguousarray(s2),
            "tailb": tb,
        })
    return in_maps, orders


def _run(x, Wq, Wk, Wv, cos, sin, trace=False):
    from concourse.bass_utils import run_bass_kernel_spmd
    nc = _get_nc()
    in_maps, orders = _prep_inputs(x, Wq, Wk, Wv, cos, sin)
    res = run_bass_kernel_spmd(nc, in_maps, list(range(8)), trace=trace)
    full = np.empty((B, T, HD), np.float32)
    for c in range(8):
        b, order = c // 2, orders[c]
        oc = res.results[c]["oT"]
        sc = res.results[c]["smv"]
        on = (oc / sc).T
        for j in range(4):
            ab = order[j]
            full[b, ab * BS:(ab + 1) * BS] = on[j * BS:(j + 1) * BS]
    return full, res


def kernel(x, Wq, Wk, Wv, cos, sin):
    return _run(x, Wq, Wk, Wv, cos, sin, trace=False)[0]


# revision 6
# speedup vs baseline: 1.0428x; 1.0428x over previous
"""Single-head causal attention with RoPE on 8 TRN2 NeuronCores.

Sharding: core c -> batch c//2, parity p = c%2 takes the interleaved
512-row q-blocks {p, p+2, p+4, p+6} of T=4096 (causal load balance).
Each core computes full K/V for its batch; no collectives.

~144us vs the 214us fp32 baseline:
- All inputs bf16, x host-packed; first two groups strip-split across
  DMA engines (one dma_start is served by a single ~20GB/s engine).
- Q^T/K^T projected directly in [d, t] layout (weights-stationary
  matmuls); RoPE on DVE via partition-offset rotate-half; V via PE
  transpose to [s, d]; per-slot tiles for fine-grained dependencies.
- Fused phases: attention score matmuls + exps + row-sum accumulation
  stream as filler between projection matmuls as soon as their
  (q-group, k-group) dependencies complete, parking probabilities in
  SBUF; each q-block's AV matmuls + output run as one burst after its
  last half arrives. This hides the exp/rowsum cost entirely under
  the PE-bound projection phase.
- Causal-exact shrinking diagonal tiles; exp bias trick for the
  other-parity tail block; output written unnormalized [d, q] + row
  sums, with the final divide and transpose on the host.
"""
import numpy as np
import ml_dtypes

B, T, C, HD = 4, 4096, 2048, 128
P = 128
NB = 8
BS = 512
NCH = 16
SCALE = float(C) ** -0.5
NEG = -1.0e9
BF = ml_dtypes.bfloat16


def build():
    import concourse.bass as bass
    import concourse.mybir as mybir
    import bass_rust
    from concourse.tile import TileContext
    from concourse.masks import make_identity

    f32 = mybir.dt.float32
    bf16 = mybir.dt.bfloat16
    EXP = mybir.ActivationFunctionType.Exp

    nc = bass.Bass()
    xg = nc.declare_dram_parameter("xg", [NB * P, NCH * BS], bf16, isOutput=False)
    wk = nc.declare_dram_parameter("wk", [P, NCH * P], bf16, isOutput=False)
    wv = nc.declare_dram_parameter("wv", [P, NCH * P], bf16, isOutput=False)
    wq = nc.declare_dram_parameter("wq", [P, NCH * P], bf16, isOutput=False)
    cs2 = nc.declare_dram_parameter("cs2", [P, T], bf16, isOutput=False)
    sn2 = nc.declare_dram_parameter("sn2", [P, T], bf16, isOutput=False)
    tailb = nc.declare_dram_parameter("tailb", [P, 1], f32, isOutput=False)
    oT = nc.declare_dram_parameter("oT", [P, 4 * BS], f32, isOutput=True)
    smv = nc.declare_dram_parameter("smv", [1, 4 * BS], f32, isOutput=True)

    NQ = 4
    CPQ = NCH // NQ

    with TileContext(nc) as tc:
        with (
            tc.tile_pool(name="const", bufs=1) as cp,
            tc.tile_pool(name="store", bufs=1) as stp,
            tc.tile_pool(name="pt", bufs=68) as ptp,
            tc.tile_pool(name="pac", bufs=4) as pap,
            tc.tile_pool(name="osb", bufs=2) as osb,
            tc.tile_pool(name="xp", bufs=2) as xp,
            tc.tile_pool(name="rp", bufs=2) as rp,
            tc.tile_pool(name="prj", bufs=1, space="PSUM") as prj,
            tc.tile_pool(name="sps", bufs=2, space="PSUM") as sps,
            tc.tile_pool(name="o2ps", bufs=1, space="PSUM") as o2ps,
            tc.tile_pool(name="smps", bufs=1, space="PSUM") as smps,
        ):
            wkt = cp.tile([P, NCH * P], bf16, tag="wkt")
            nc.sync.dma_start(wkt[:, 0:NCH * P // 2], wk[:, 0:NCH * P // 2])
            nc.sync.dma_start(wkt[:, NCH * P // 2:], wk[:, NCH * P // 2:])
            wvt = cp.tile([P, NCH * P], bf16, tag="wvt")
            nc.sync.dma_start(wvt[:, 0:NCH * P // 2], wv[:, 0:NCH * P // 2])
            nc.sync.dma_start(wvt[:, NCH * P // 2:], wv[:, NCH * P // 2:])
            wqt = cp.tile([P, NCH * P], bf16, tag="wqt")
            nc.sync.dma_start(wqt[:, 0:NCH * P // 2], wq[:, 0:NCH * P // 2])
            nc.sync.dma_start(wqt[:, NCH * P // 2:], wq[:, NCH * P // 2:])
            cst = cp.tile([P, T], bf16, tag="cst")
            nc.scalar.dma_start(cst[:, 0:T // 2], cs2[:, 0:T // 2])
            nc.scalar.dma_start(cst[:, T // 2:], cs2[:, T // 2:])
            snt = cp.tile([P, T], bf16, tag="snt")
            nc.scalar.dma_start(snt[:, 0:T // 2], sn2[:, 0:T // 2])
            nc.scalar.dma_start(snt[:, T // 2:], sn2[:, T // 2:])
            tb = cp.tile([P, 1], f32, tag="tb")
            nc.scalar.dma_start(tb[:], tailb[:])

            ident = cp.tile([P, P], bf16, tag="ident")
            make_identity(nc, ident[:])
            tri = cp.tile([P, P], bf16, tag="tri")
            nc.gpsimd.memset(tri[:], 0.0)
            nc.gpsimd.affine_select(
                out=tri[:], in_=tri[:],
                compare_op=mybir.AluOpType.is_gt,
                fill=1.0, base=0,
                pattern=[[-1, P]], channel_multiplier=1,
            )
            ones = cp.tile([P, 1], bf16, tag="ones")
            nc.gpsimd.memset(ones[:], 1.0)

            kTs = [stp.tile([P, BS], bf16, tag=f"kT{s}", name=f"kTs{s}") for s in range(8)]
            vsbs = [stp.tile([P, BS], bf16, tag=f"vs{s}", name=f"vsbs{s}") for s in range(8)]
            qTs = [stp.tile([P, BS], bf16, tag=f"qT{s}", name=f"qTs{s}") for s in range(4)]

            # ---------- attention pair bookkeeping ----------
            # per j: halves in order; each half:
            #   (si, kind, st, off, w)   off/w only shrink for diag
            def j_halves(j):
                hs = []
                for st in range(4):
                    hs.append((j, "diag", st, st * P, BS - st * P))
                for s in list(range(j)) + [4 + s for s in range(j)]:
                    for st in range(4):
                        hs.append((s, "full", st, 0, BS))
                for st in range(4):
                    hs.append((4 + j, "tail", st, 0, BS))
                return hs

            # group position in plain order = group index
            def ready_pos(j, si):
                return max(j, si)   # needs qTs[j] (group j), kTs/vsbs[si]

            pending = []    # (j, half_idx, half_desc) in j-major order
            for j in range(4):
                for hi, h in enumerate(j_halves(j)):
                    pending.append([j, hi, h, False])   # emitted flag
            parked = {j: [] for j in range(4)}   # (half_desc, Pt tile)
            pacs = {}
            npend = {j: len(j_halves(j)) for j in range(4)}
            avdone = set()
            done_groups = 0    # groups fully emitted so far

            neng = [0]

            def emit_half(j, hi, h):
                si, kind, st, off, w = h
                scol = st * P
                Sp = sps.tile([P, BS], f32, tag="S")
                Pt = ptp.tile([P, BS], bf16, tag="Pt")
                nc.tensor.matmul(
                    Sp[:, 0:w], kTs[si][:, scol:scol + P],
                    qTs[j][:, off:BS], start=True, stop=True)
                bias = tb[:, 0:1] if kind == "tail" else 0.0
                nc.scalar.activation(Pt[:, 0:w], Sp[:, 0:w], EXP,
                                     bias=bias, scale=SCALE)
                if kind == "diag":
                    nc.vector.tensor_mul(Pt[:, 0:P], Pt[:, 0:P], tri[:])
                # row-sum accumulate into pac(j): alternate DVE/GpSimd
                if j not in pacs:
                    pacs[j] = pap.tile([P, BS], f32, tag="pac", name=f"pac{j}")
                    assert kind == "diag" and st == 0
                    nc.vector.tensor_copy(pacs[j][:], Pt[:, 0:BS])
                else:
                    nc.vector.tensor_add(pacs[j][:, off:BS],
                                         pacs[j][:, off:BS], Pt[:, 0:w])
                parked[j].append((h, Pt))

            def filler():
                # emit one ready, unemitted attention half
                for ent in pending:
                    j, hi, h, em = ent
                    if em or j in avdone:
                        continue
                    if ready_pos(j, h[0]) < done_groups:
                        ent[3] = True
                        emit_half(j, hi, h)
                        return True
                return False

            def flush_ready_js():
                for j in range(4):
                    if j in avdone or len(parked[j]) < npend[j]:
                        continue
                    avdone.add(j)
                    o2 = o2ps.tile([P, BS], f32, tag="o2")
                    nh = len(parked[j])
                    for ix, (h, Pt) in enumerate(parked[j]):
                        si, kind, st, off, w = h
                        scol = st * P
                        nc.tensor.matmul(
                            o2[:, off:BS], vsbs[si][:, scol:scol + P],
                            Pt[:, 0:w],
                            start=(ix == 0), stop=(ix == nh - 1))
                    parked[j].clear()
                    pacbf = pap.tile([P, BS], bf16, tag="pacbf")
                    nc.scalar.copy(pacbf[:], pacs[j][:])
                    sm = smps.tile([1, BS], f32, tag="sm")
                    nc.tensor.matmul(sm[:], ones[:], pacbf[:],
                                     start=True, stop=True)
                    qsl = slice(j * BS, (j + 1) * BS)
                    o2sb = osb.tile([P, BS], f32, tag="o2sb")
                    nc.vector.tensor_copy(o2sb[:], o2[:])
                    nc.sync.dma_start(oT[:, qsl], o2sb[:])
                    smsb = osb.tile([1, BS], f32, tag="smsb")
                    nc.scalar.copy(smsb[:], sm[:])
                    nc.sync.dma_start(smv[:, qsl], smsb[:])

            # ---------- fused main loop ----------
            for g in range(NB):
                gs = slice(g * BS, (g + 1) * BS)
                xts = []
                for q4 in range(NQ):
                    xt = xp.tile([P, CPQ * BS], bf16, tag=f"xg{q4}")
                    c0 = q4 * CPQ * BS
                    if g < 2:
                        for s4 in range(4):
                            r0 = g * P + s4 * 32
                            eng = nc.sync if s4 % 2 == 0 else nc.scalar
                            eng.dma_start(
                                xt[s4 * 32:(s4 + 1) * 32, :],
                                xg[r0:r0 + 32, c0:c0 + CPQ * BS])
                    else:
                        nc.sync.dma_start(
                            xt[:], xg[g * P:(g + 1) * P, c0:c0 + CPQ * BS])
                    xts.append(xt)

                def proj(wt, tag):
                    pp = prj.tile([P, BS], f32, tag=tag)
                    for ci in range(NCH):
                        nc.tensor.matmul(
                            pp[:], wt[:, ci * P:(ci + 1) * P],
                            xts[ci // CPQ][:, (ci % CPQ) * BS:
                                           (ci % CPQ + 1) * BS],
                            start=(ci == 0), stop=(ci == NCH - 1))
                        if ci % 4 == 3:
                            filler()
                    return pp

                def rope(pp, dst):
                    H = 64
                    m1 = rp.tile([P, BS], bf16, tag="m1")
                    nc.vector.tensor_mul(m1[:], pp[:], cst[:, gs])
                    rot = rp.tile([P, BS], bf16, tag="rot")
                    nc.vector.tensor_mul(rot[0:H, :], pp[H:P, :],
                                         snt[0:H, gs])
                    nc.vector.tensor_mul(rot[H:P, :], pp[0:H, :],
                                         snt[H:P, gs])
                    nc.vector.tensor_add(dst, m1[:], rot[:])

                kp = proj(wkt, "kps")
                rope(kp, kTs[g][:])
                vp = proj(wvt, "vps")
                vsbh = rp.tile([P, BS], bf16, tag="vsbh")
                nc.scalar.copy(vsbh[:], vp[:])
                vtp = prj.tile([P, BS], bf16, tag="vtp")
                for k in range(4):
                    ks = slice(k * P, (k + 1) * P)
                    nc.tensor.transpose(vtp[:, ks], vsbh[:, ks], ident[:])
                    nc.scalar.copy(vsbs[g][:, ks], vtp[:, ks])
                    filler()
                if g < 4:
                    qp = proj(wqt, "qps")
                    rope(qp, qTs[g][:])
                done_groups = g + 1
                # drain a few extra fillers between groups, then AV bursts
                for _ in range(6):
                    if not filler():
                        break
                flush_ready_js()

            # tail: whatever is left (j3's slot-7 halves + AV bursts)
            while filler():
                pass
            flush_ready_js()

    bass_rust.generate_event_semaphores(nc)
    return nc


_CACHE = {}


def _get_nc():
    if "nc" not in _CACHE:
        _CACHE["nc"] = build()
    return _CACHE["nc"]


def _prep_inputs(x, Wq, Wk, Wv, cos, sin):
    x = np.asarray(x, np.float32)
    Wq = np.asarray(Wq, np.float32)
    Wk = np.asarray(Wk, np.float32)
    Wv = np.asarray(Wv, np.float32)
    cos = np.asarray(cos, np.float32)
    sin = np.asarray(sin, np.float32)
    perm = np.concatenate([np.arange(0, HD, 2), np.arange(1, HD, 2)])

    def packw(wt):
        return np.ascontiguousarray(
            wt.reshape(NCH, P, HD).transpose(1, 0, 2).reshape(P, NCH * HD))

    wq = packw(Wq[perm].T.astype(BF))
    wk = packw(Wk[perm].T.astype(BF))
    wv = packw(Wv.T.astype(BF))
    cosT = cos.T.astype(np.float32)
    sinT = sin.T.astype(np.float32)
    cs2f = np.concatenate([cosT, cosT], axis=0)
    sn2f = np.concatenate([-sinT, sinT], axis=0)
    in_maps = []
    orders = []
    for c in range(8):
        b, par = c // 2, c % 2
        order = [par, par + 2, par + 4, par + 6,
                 1 - par, 3 - par, 5 - par, 7 - par]
        orders.append(order)
        xb = np.asarray(x[b], np.float32)
        xgl = np.empty((NB, P, NCH, BS), BF)
        c2 = np.empty((P, T), BF)
        s2 = np.empty((P, T), BF)
        for sl, ab in enumerate(order):
            seg = xb[ab * BS:(ab + 1) * BS].T.astype(BF)
            xgl[sl] = seg.reshape(NCH, P, BS).transpose(1, 0, 2)
            dst = slice(sl * BS, (sl + 1) * BS)
            src = slice(ab * BS, (ab + 1) * BS)
            c2[:, dst] = cs2f[:, src].astype(BF)
            s2[:, dst] = sn2f[:, src].astype(BF)
        tb = np.full((P, 1), NEG if par == 0 else 0.0, np.float32)
        in_maps.append({
            "xg": np.ascontiguousarray(xgl.reshape(NB * P, NCH * BS)),
            "wk": wk, "wv": wv, "wq": wq,
            "cs2": np.ascontiguousarray(c2),
            "sn2": np.ascontiguousarray(s2),
            "tailb": tb,
        })
    return in_maps, orders


def _run(x, Wq, Wk, Wv, cos, sin, trace=False):
    from concourse.bass_utils import run_bass_kernel_spmd
    nc = _get_nc()
    in_maps, orders = _prep_inputs(x, Wq, Wk, Wv, cos, sin)
    res = run_bass_kernel_spmd(nc, in_maps, list(range(8)), trace=trace)
    full = np.empty((B, T, HD), np.float32)
    for c in range(8):
        b, order = c // 2, orders[c]
        oc = res.results[c]["oT"]
        sc = res.results[c]["smv"]
        on = (oc / sc).T
        for j in range(4):
            ab = order[j]
            full[b, ab * BS:(ab + 1) * BS] = on[j * BS:(j + 1) * BS]
    return full, res


def kernel(x, Wq, Wk, Wv, cos, sin):
    return _run(x, Wq, Wk, Wv, cos, sin, trace=False)[0]
